# revision 1
# baseline (speedup 1.0000x reference)
"""LoRA Linear kernel for 8x TRN2 NeuronCores (Bass/Tile).

Computes  y = x @ W^T + b + 2.0 * ((x @ A^T) @ B^T)   for
  x [4, 2048, 4096] f32, W [4096, 4096], b [4096], A [16, 4096], B [4096, 16].

Strategy:
  - Data-parallel over tokens: 8192 tokens -> 1024 per core.
  - Host-side prep: transpose x and W to contraction-major layout and cast to
    bf16 (fp32 accumulate in PSUM), so the device does zero transposes.
  - LoRA rank-16 path and the bias are folded into the same PSUM accumulation
    as the base matmul: a K=16 matmul against xa^T and a K=1 matmul of
    ones^T @ b.  PSUM is drained via an ACT copy to SBUF, then DMA to DRAM.
  - Each SBUF tile has exactly one producer proc: Ldweights/TensorCopy can
    encode only a single semaphore wait in walrus codegen.
"""

import os

import numpy as np
import ml_dtypes

_BF16 = ml_dtypes.bfloat16

# Problem constants (hardcoded per harness contract).
_B, _S, _D, _O, _R = 4, 2048, 4096, 4096, 16
_T = _B * _S          # 8192 tokens
_NCORES = 8
_TC = _T // _NCORES   # 1024 tokens per core

P = 128
DS = _D // P          # 32 contraction subtiles
NTT = _TC // P        # 8 t-tiles per core
OBW = 512             # o-block width (one PSUM bank of f32)
NOB = _O // OBW       # 8 o-blocks
XA_CH = 512           # token chunk for the xa matmul
NXA = _TC // XA_CH    # 2

_cache = {}

# Set by kernel() when KERNEL_TRACE=1; read by test.py for exec_time_ns.
LAST_RESULT = None


def _build_module():
    import concourse.bass as bass
    import concourse.bacc as bacc
    import concourse.mybir as mybir
    import concourse.tile as tile
    from concourse.bass import ts

    bf16 = mybir.dt.bfloat16
    f32 = mybir.dt.float32

    nc = bacc.Bacc("TRN2", target_bir_lowering=False, debug=False)
    xT_d = nc.dram_tensor("xT", [_D, _TC], bf16, kind="ExternalInput")
    WT_d = nc.dram_tensor("WT", [_D, _O], bf16, kind="ExternalInput")
    AT_d = nc.dram_tensor("AT", [_D, _R], bf16, kind="ExternalInput")
    BT_d = nc.dram_tensor("BT", [_R, _O], bf16, kind="ExternalInput")
    bvec_d = nc.dram_tensor("bvec", [1, _O], bf16, kind="ExternalInput")
    ones_d = nc.dram_tensor("ones", [1, _TC], bf16, kind="ExternalInput")
    out_d = nc.dram_tensor("out", [_TC, _O], f32, kind="ExternalOutput")

    xT_r = xT_d[:, :].rearrange("(ds p) t -> p ds t", p=P)
    WT_r = WT_d[:, :].rearrange("(ds p) o -> p ds o", p=P)
    AT_r = AT_d[:, :].rearrange("(ds p) r -> p ds r", p=P)

    with tile.TileContext(nc) as tc:
        with (
            tc.tile_pool(name="const", bufs=1) as cpool,
            tc.tile_pool(name="wpool", bufs=2) as wpool,
            tc.tile_pool(name="opool", bufs=6) as opool,
            tc.tile_pool(name="ps_mm", bufs=4, space="PSUM") as ps_pool,
            tc.tile_pool(name="ps_xa", bufs=2, space="PSUM") as ps_xa_pool,
        ):
            xT_sb = cpool.tile([P, DS, _TC], bf16)     # 64KB/partition
            AT_sb = cpool.tile([P, DS, _R], bf16)
            BT_sb = cpool.tile([_R, _O], bf16)
            b_sb = cpool.tile([1, _O], bf16)
            ones_sb = cpool.tile([1, _TC], bf16)
            xaT_sb = cpool.tile([_R, _TC], bf16)

            half = _TC // 2
            for h in range(2):
                sl = slice(h * half, (h + 1) * half)
                nc.sync.dma_start(xT_sb[:, :, sl], xT_r[:, :, sl])
            nc.sync.dma_start(AT_sb[:], AT_r[:])
            nc.sync.dma_start(BT_sb[:], BT_d[:, :])
            nc.sync.dma_start(b_sb[:], bvec_d[:, :])
            nc.sync.dma_start(ones_sb[:], ones_d[:, :])

            # xa^T[r, t] = sum_d A^T[d, r] * x^T[d, t], accumulated in PSUM.
            for cx in range(NXA):
                ps_xa = ps_xa_pool.tile([_R, XA_CH], f32)
                for ds in range(DS):
                    nc.tensor.matmul(
                        ps_xa[:],
                        AT_sb[:, ds, :],
                        xT_sb[:, ds, ts(cx, XA_CH)],
                        start=(ds == 0),
                        stop=(ds == DS - 1),
                    )
                nc.vector.tensor_copy(xaT_sb[:, ts(cx, XA_CH)], ps_xa[:])

            for ob in range(NOB):
                WT_blk = wpool.tile([P, DS, OBW], bf16)
                nc.sync.dma_start(WT_blk[:], WT_r[:, :, ts(ob, OBW)])
                for tt in range(NTT):
                    ps = ps_pool.tile([P, OBW], f32)
                    for ds in range(DS):
                        nc.tensor.matmul(
                            ps[:],
                            xT_sb[:, ds, ts(tt, P)],
                            WT_blk[:, ds, :],
                            start=(ds == 0),
                            stop=False,
                        )
                    # LoRA: xa^T.T @ (2 B^T), K=16
                    nc.tensor.matmul(
                        ps[:],
                        xaT_sb[:, ts(tt, P)],
                        BT_sb[:, ts(ob, OBW)],
                        start=False,
                        stop=False,
                    )
                    # bias: ones^T @ b, K=1
                    nc.tensor.matmul(
                        ps[:],
                        ones_sb[:, ts(tt, P)],
                        b_sb[:, ts(ob, OBW)],
                        start=False,
                        stop=True,
                    )
                    ot = opool.tile([P, OBW], f32)
                    nc.scalar.copy(ot[:], ps[:])
                    nc.sync.dma_start(out_d[ts(tt, P), ts(ob, OBW)], ot[:])
    nc.compile()
    return nc


def kernel(x, W, b, lora_A, lora_B):
    global LAST_RESULT
    from concourse.bass_utils import run_bass_kernel_spmd

    if "nc" not in _cache:
        _cache["nc"] = _build_module()
    nc = _cache["nc"]

    xf = np.ascontiguousarray(x.reshape(_T, _D)).astype(_BF16)
    xT = np.ascontiguousarray(xf.T)                              # [D, T]
    WT = np.ascontiguousarray(W.astype(_BF16).T)                 # [D, O]
    AT = np.ascontiguousarray(lora_A.astype(_BF16).T)            # [D, R]
    BT = np.ascontiguousarray((2.0 * lora_B).astype(_BF16).T)    # [R, O]

    in_maps = []
    for c in range(_NCORES):
        in_maps.append(
            {
                "xT": np.ascontiguousarray(xT[:, c * _TC : (c + 1) * _TC]),
                "WT": WT,
                "AT": AT,
                "BT": BT,
                "bvec": b.astype(_BF16)[None, :],
                "ones": np.ones((1, _TC), dtype=_BF16),
            }
        )

    trace = os.environ.get("KERNEL_TRACE", "0") == "1"
    res = run_bass_kernel_spmd(
        nc,
        in_maps,
        core_ids=list(range(_NCORES)),
        trace=trace,
    )
    LAST_RESULT = res

    out = np.concatenate([r["out"] for r in res.results], axis=0)
    return out.reshape(_B, _S, _O).astype(np.float32, copy=False)



# revision 10
# speedup vs baseline: 1.2557x; 1.2557x over previous
"""LoRA Linear kernel for 8x TRN2 NeuronCores (Bass/Tile).

Computes  y = x @ W^T + b + 2.0 * ((x @ A^T) @ B^T)   for
  x [4, 2048, 4096] f32, W [4096, 4096], b [4096], A [16, 4096], B [4096, 16].

Strategy (v2):
  - Data-parallel over tokens: 8192 tokens -> 1024 per core.
  - Stationary operand is the W o-tile [128d, 128o]; the moving operand
    sweeps tokens, so one Ldweights serves two N=512 matmuls (the baseline
    was 1 Ldweights per matmul, which exposed ~43ns per pair).
  - Output computed as [O, TC] per core (o on partitions); host transposes.
  - Bias is folded into the LoRA matmul: stationary [17, o128] =
    [2*B^T; b], moving [17, t512] = [xa^T; ones].  One K=17 matmul per
    (o-tile, t-chunk) finishes each PSUM accumulation group.
  - W is host-prepacked per o-tile so each DMA line is 8KB contiguous.
  - DMA queues: sync = x/A/Baug in, scalar = W stream in, vector = out.
"""

import os

import numpy as np
import ml_dtypes

_BF16 = ml_dtypes.bfloat16

# Problem constants (hardcoded per harness contract).
_B, _S, _D, _O, _R = 4, 2048, 4096, 4096, 16
_T = _B * _S          # 8192 tokens
_NCORES = 8
_TC = _T // _NCORES   # 1024 tokens per core

P = 128
DS = _D // P          # 32 contraction subtiles
NOT = _O // P         # 32 o-tiles
TCH = 512             # token chunk (moving N)
NCH = _TC // TCH      # 2 chunks per core
RA = _R + 1           # lora rows + bias row

_cache = {}

# Set by kernel() when KERNEL_TRACE=1; read by test.py for exec_time_ns.
LAST_RESULT = None


def _build_module():
    import concourse.bass as bass
    import concourse.bacc as bacc
    import concourse.mybir as mybir
    import concourse.tile as tile
    from concourse.bass import ts

    bf16 = mybir.dt.bfloat16
    f32 = mybir.dt.float32

    nc = bacc.Bacc("TRN2", target_bir_lowering=False, debug=False)
    x0_d = nc.dram_tensor("x0", [P, DS, TCH], bf16, kind="ExternalInput")
    x1_d = nc.dram_tensor("x1", [P, DS, TCH], bf16, kind="ExternalInput")
    Wp_d = nc.dram_tensor("Wp", [NOT * P, DS, P], bf16, kind="ExternalInput")
    ATp_d = nc.dram_tensor("ATp", [P, DS, _R], bf16, kind="ExternalInput")
    Baug_d = nc.dram_tensor("Baug", [RA, _O], bf16, kind="ExternalInput")
    ones_d = nc.dram_tensor("ones", [1, _TC], bf16, kind="ExternalInput")
    out_d = nc.dram_tensor("out", [_O, _TC], f32, kind="ExternalOutput")

    with tile.TileContext(nc) as tc:
        with (
            tc.tile_pool(name="const", bufs=1) as cpool,
            tc.tile_pool(name="wpool", bufs=6) as wpool,
            tc.tile_pool(name="opool", bufs=3) as opool,
            tc.tile_pool(name="ps_mm", bufs=2, space="PSUM") as ps_pool,
            tc.tile_pool(name="ps_xa", bufs=2, space="PSUM") as ps_xa_pool,
        ):
            AT_sb = cpool.tile([P, DS, _R], bf16)
            x_sb = [
                cpool.tile([P, DS, TCH], bf16, name=f"x_sb{c}") for c in range(NCH)
            ]
            Baug_sb = cpool.tile([RA, _O], bf16)
            xa_aug = cpool.tile([RA, _TC], bf16)

            nc.sync.dma_start(xa_aug[_R : _R + 1, :], ones_d[:, :])
            nc.sync.dma_start(AT_sb[:], ATp_d[:, :, :])
            nc.sync.dma_start(x_sb[0][:], x0_d[:, :, :])
            nc.sync.dma_start(x_sb[1][:], x1_d[:, :, :])
            nc.sync.dma_start(Baug_sb[:], Baug_d[:, :])

            # xa^T[r, t] = sum_ds A^T[ds, r].T @ x^T[ds, t]
            for c in range(NCH):
                ps_xa = ps_xa_pool.tile([_R, TCH], f32)
                for ds in range(DS):
                    nc.tensor.matmul(
                        ps_xa[:],
                        AT_sb[:, ds, :],
                        x_sb[c][:, ds, :],
                        start=(ds == 0),
                        stop=(ds == DS - 1),
                    )
                nc.vector.tensor_copy(xa_aug[0:_R, ts(c, TCH)], ps_xa[:])

            for ot in range(NOT):
                Wt = wpool.tile([P, DS, P], bf16)
                nc.scalar.dma_start(Wt[:], Wp_d[ts(ot, P), :, :])
                ps = [
                    ps_pool.tile([P, TCH], f32, name=f"ps{c}") for c in range(NCH)
                ]
                for ds in range(DS):
                    for c in range(NCH):
                        nc.tensor.matmul(
                            ps[c][:],
                            Wt[:, ds, :],
                            x_sb[c][:, ds, :],
                            start=(ds == 0),
                            stop=False,
                        )
                # LoRA + bias: [2B^T; b][:, ot].T @ [xa^T; ones] , K=17
                for c in range(NCH):
                    nc.tensor.matmul(
                        ps[c][:],
                        Baug_sb[:, ts(ot, P)],
                        xa_aug[:, ts(c, TCH)],
                        start=False,
                        stop=True,
                    )
                ot_sb = opool.tile([P, _TC], f32)
                for c in range(NCH):
                    nc.scalar.copy(ot_sb[:, ts(c, TCH)], ps[c][:])
                nc.sync.dma_start(out_d[ts(ot, P), :], ot_sb[:])

    _dedup_ldweights(nc, mybir)
    nc.compile()
    return nc


def _dedup_ldweights(nc, mybir):
    """Drop PE Ldweights that reload the stationary already in the array.

    The tile pass lowers every matmul to an Ldweights+Matmult pair even when
    consecutive matmuls share the stationary operand.  The redundant reload
    costs PE cycles (~46ns exposed per pair at N=512).  Weights persist in
    the array across Matmults, so a back-to-back identical Ldweights with no
    semaphore activity is dead.
    """
    n_drop = 0
    for fn in nc.m.functions:
        for blk in fn.blocks:
            insts = blk.instructions
            new = []
            prev_key = None
            for inst in insts:
                if inst.engine != mybir.EngineType.PE:
                    new.append(inst)
                    continue
                if isinstance(inst, mybir.InstLdweights):
                    key = str(inst.ins[0])
                    if (
                        key == prev_key
                        and not inst.has_wait()
                        and not inst.has_update()
                    ):
                        n_drop += 1
                        continue
                    prev_key = key
                elif isinstance(inst, mybir.InstMatmult):
                    if inst.is_transpose:
                        prev_key = None
                elif isinstance(inst, mybir.InstEventSemaphore):
                    pass
                else:
                    prev_key = None
                new.append(inst)
            if n_drop:
                blk.instructions = new
    if os.environ.get("KERNEL_DEBUG"):
        print(f"_dedup_ldweights: dropped {n_drop}")


def kernel(x, W, b, lora_A, lora_B):
    global LAST_RESULT
    from concourse.bass_utils import run_bass_kernel_spmd

    if "nc" not in _cache:
        _cache["nc"] = _build_module()
    nc = _cache["nc"]

    xf = np.ascontiguousarray(x.reshape(_T, _D)).astype(_BF16)
    xT = np.ascontiguousarray(xf.T)                              # [D, T]
    # [D, T] -> [p, ds, T] so each DMA line is contiguous per partition
    xprep = np.ascontiguousarray(xT.reshape(DS, P, _T).transpose(1, 0, 2))
    WT = W.astype(_BF16).T                                       # [D, O]
    # [ds, p, ot, o] -> [ot, p, ds, o] -> [ot*p, ds, o]: 8KB contiguous lines
    Wprep = np.ascontiguousarray(
        WT.reshape(DS, P, NOT, P).transpose(2, 1, 0, 3)
    ).reshape(NOT * P, DS, P)
    ATprep = np.ascontiguousarray(
        lora_A.astype(_BF16).T.reshape(DS, P, _R).transpose(1, 0, 2)
    )
    Baug = np.concatenate(
        [(2.0 * lora_B).astype(_BF16).T, b.astype(_BF16)[None, :]], axis=0
    )  # [17, O]

    in_maps = []
    for c in range(_NCORES):
        t0 = c * _TC
        in_maps.append(
            {
                "x0": np.ascontiguousarray(xprep[:, :, t0 : t0 + TCH]),
                "x1": np.ascontiguousarray(xprep[:, :, t0 + TCH : t0 + 2 * TCH]),
                "Wp": Wprep,
                "ATp": ATprep,
                "Baug": Baug,
                "ones": np.ones((1, _TC), dtype=_BF16),
            }
        )

    trace = os.environ.get("KERNEL_TRACE", "0") == "1"
    res = run_bass_kernel_spmd(
        nc,
        in_maps,
        core_ids=list(range(_NCORES)),
        trace=trace,
    )
    LAST_RESULT = res

    out = np.empty((_T, _O), dtype=np.float32)
    for c, r in enumerate(res.results):
        out[c * _TC : (c + 1) * _TC, :] = r["out"].T
    return out.reshape(_B, _S, _O)


# revision 12
# speedup vs baseline: 1.2622x; 1.0051x over previous
"""LoRA Linear kernel for 8x TRN2 NeuronCores (Bass/Tile).

Computes  y = x @ W^T + b + 2.0 * ((x @ A^T) @ B^T)   for
  x [4, 2048, 4096] f32, W [4096, 4096], b [4096], A [16, 4096], B [4096, 16].

Strategy (v2):
  - Data-parallel over tokens: 8192 tokens -> 1024 per core.
  - Stationary operand is the W o-tile [128d, 128o]; the moving operand
    sweeps tokens, so one Ldweights serves two N=512 matmuls (the baseline
    was 1 Ldweights per matmul, which exposed ~43ns per pair).
  - Output computed as [O, TC] per core (o on partitions); host transposes.
  - Bias is folded into the LoRA matmul: stationary [17, o128] =
    [2*B^T; b], moving [17, t512] = [xa^T; ones].  One K=17 matmul per
    (o-tile, t-chunk) finishes each PSUM accumulation group.
  - W is host-prepacked per o-tile so each DMA line is 8KB contiguous.
  - DMA queues: sync = x/A/Baug in, scalar = W stream in, vector = out.
"""

import os

import numpy as np
import ml_dtypes

_BF16 = ml_dtypes.bfloat16

# Problem constants (hardcoded per harness contract).
_B, _S, _D, _O, _R = 4, 2048, 4096, 4096, 16
_T = _B * _S          # 8192 tokens
_NCORES = 8
_TC = _T // _NCORES   # 1024 tokens per core

P = 128
DS = _D // P          # 32 contraction subtiles
NOT = _O // P         # 32 o-tiles
TCH = 512             # token chunk (moving N)
NCH = _TC // TCH      # 2 chunks per core
RA = _R + 1           # lora rows + bias row

_cache = {}

# Set by kernel() when KERNEL_TRACE=1; read by test.py for exec_time_ns.
LAST_RESULT = None


def _build_module():
    import concourse.bass as bass
    import concourse.bacc as bacc
    import concourse.mybir as mybir
    import concourse.tile as tile
    from concourse.bass import ts

    bf16 = mybir.dt.bfloat16
    f32 = mybir.dt.float32

    nc = bacc.Bacc("TRN2", target_bir_lowering=False, debug=False)
    x0_d = nc.dram_tensor("x0", [P, DS, TCH], bf16, kind="ExternalInput")
    x1_d = nc.dram_tensor("x1", [P, DS, TCH], bf16, kind="ExternalInput")
    Wp_d = nc.dram_tensor("Wp", [NOT * P, DS, P], bf16, kind="ExternalInput")
    ATp_d = nc.dram_tensor("ATp", [P, DS, _R], bf16, kind="ExternalInput")
    Baug_d = nc.dram_tensor("Baug", [RA, _O], bf16, kind="ExternalInput")
    ones_d = nc.dram_tensor("ones", [1, _TC], bf16, kind="ExternalInput")
    out_d = nc.dram_tensor("out", [_O, _TC], f32, kind="ExternalOutput")

    with tile.TileContext(nc) as tc:
        with (
            tc.tile_pool(name="const", bufs=1) as cpool,
            tc.tile_pool(name="wpool", bufs=6) as wpool,
            tc.tile_pool(name="opool", bufs=3) as opool,
            tc.tile_pool(name="ps_mm", bufs=2, space="PSUM") as ps_pool,
            tc.tile_pool(name="ps_xa", bufs=2, space="PSUM") as ps_xa_pool,
        ):
            AT_sb = cpool.tile([P, DS, _R], bf16)
            x_sb = [
                cpool.tile([P, DS, TCH], bf16, name=f"x_sb{c}") for c in range(NCH)
            ]
            Baug_sb = cpool.tile([RA, _O], bf16)
            xa_aug = cpool.tile([RA, _TC], bf16)

            # Split each x chunk across both HWDGE queues (ds halves) so the
            # first xa matmul can start ~6.5us in instead of ~26us.
            H = DS // 2
            nc.sync.dma_start(AT_sb[:], ATp_d[:, :, :])
            nc.sync.dma_start(x_sb[0][:, 0:H, :], x0_d[:, 0:H, :])
            nc.scalar.dma_start(x_sb[0][:, H:DS, :], x0_d[:, H:DS, :])
            nc.sync.dma_start(x_sb[1][:, 0:H, :], x1_d[:, 0:H, :])
            nc.scalar.dma_start(x_sb[1][:, H:DS, :], x1_d[:, H:DS, :])
            nc.sync.dma_start(xa_aug[_R : _R + 1, :], ones_d[:, :])
            nc.sync.dma_start(Baug_sb[:], Baug_d[:, :])

            # xa^T[r, t] = sum_ds A^T[ds, r].T @ x^T[ds, t]
            for c in range(NCH):
                ps_xa = ps_xa_pool.tile([_R, TCH], f32)
                for ds in range(DS):
                    nc.tensor.matmul(
                        ps_xa[:],
                        AT_sb[:, ds, :],
                        x_sb[c][:, ds, :],
                        start=(ds == 0),
                        stop=(ds == DS - 1),
                    )
                nc.vector.tensor_copy(xa_aug[0:_R, ts(c, TCH)], ps_xa[:])

            for ot in range(NOT):
                Wt = wpool.tile([P, DS, P], bf16)
                nc.scalar.dma_start(Wt[:], Wp_d[ts(ot, P), :, :])
                ps = [
                    ps_pool.tile([P, TCH], f32, name=f"ps{c}") for c in range(NCH)
                ]
                for ds in range(DS):
                    for c in range(NCH):
                        nc.tensor.matmul(
                            ps[c][:],
                            Wt[:, ds, :],
                            x_sb[c][:, ds, :],
                            start=(ds == 0),
                            stop=False,
                        )
                # LoRA + bias: [2B^T; b][:, ot].T @ [xa^T; ones] , K=17
                for c in range(NCH):
                    nc.tensor.matmul(
                        ps[c][:],
                        Baug_sb[:, ts(ot, P)],
                        xa_aug[:, ts(c, TCH)],
                        start=False,
                        stop=True,
                    )
                # Parallel drains (ACT + DVE, different PSUM banks) and
                # per-chunk out DMAs on alternating queues: short tail.
                ot_sb0 = opool.tile([P, TCH], f32, name="ot_sb0")
                ot_sb1 = opool.tile([P, TCH], f32, name="ot_sb1")
                nc.scalar.copy(ot_sb0[:], ps[0][:])
                nc.vector.tensor_copy(ot_sb1[:], ps[1][:])
                nc.sync.dma_start(out_d[ts(ot, P), 0:TCH], ot_sb0[:])
                nc.scalar.dma_start(out_d[ts(ot, P), TCH : 2 * TCH], ot_sb1[:])

    _dedup_ldweights(nc, mybir)
    nc.compile()
    return nc


def _dedup_ldweights(nc, mybir):
    """Drop PE Ldweights that reload the stationary already in the array.

    The tile pass lowers every matmul to an Ldweights+Matmult pair even when
    consecutive matmuls share the stationary operand.  The redundant reload
    costs PE cycles (~46ns exposed per pair at N=512).  Weights persist in
    the array across Matmults, so a back-to-back identical Ldweights with no
    semaphore activity is dead.
    """
    n_drop = 0
    for fn in nc.m.functions:
        for blk in fn.blocks:
            insts = blk.instructions
            new = []
            prev_key = None
            for inst in insts:
                if inst.engine != mybir.EngineType.PE:
                    new.append(inst)
                    continue
                if isinstance(inst, mybir.InstLdweights):
                    key = str(inst.ins[0])
                    if (
                        key == prev_key
                        and not inst.has_wait()
                        and not inst.has_update()
                    ):
                        n_drop += 1
                        continue
                    prev_key = key
                elif isinstance(inst, mybir.InstMatmult):
                    if inst.is_transpose:
                        prev_key = None
                elif isinstance(inst, mybir.InstEventSemaphore):
                    pass
                else:
                    prev_key = None
                new.append(inst)
            if n_drop:
                blk.instructions = new
    if os.environ.get("KERNEL_DEBUG"):
        print(f"_dedup_ldweights: dropped {n_drop}")


def kernel(x, W, b, lora_A, lora_B):
    global LAST_RESULT
    from concourse.bass_utils import run_bass_kernel_spmd

    if "nc" not in _cache:
        _cache["nc"] = _build_module()
    nc = _cache["nc"]

    xf = np.ascontiguousarray(x.reshape(_T, _D)).astype(_BF16)
    xT = np.ascontiguousarray(xf.T)                              # [D, T]
    # [D, T] -> [p, ds, T] so each DMA line is contiguous per partition
    xprep = np.ascontiguousarray(xT.reshape(DS, P, _T).transpose(1, 0, 2))
    WT = W.astype(_BF16).T                                       # [D, O]
    # [ds, p, ot, o] -> [ot, p, ds, o] -> [ot*p, ds, o]: 8KB contiguous lines
    Wprep = np.ascontiguousarray(
        WT.reshape(DS, P, NOT, P).transpose(2, 1, 0, 3)
    ).reshape(NOT * P, DS, P)
    ATprep = np.ascontiguousarray(
        lora_A.astype(_BF16).T.reshape(DS, P, _R).transpose(1, 0, 2)
    )
    Baug = np.concatenate(
        [(2.0 * lora_B).astype(_BF16).T, b.astype(_BF16)[None, :]], axis=0
    )  # [17, O]

    in_maps = []
    for c in range(_NCORES):
        t0 = c * _TC
        in_maps.append(
            {
                "x0": np.ascontiguousarray(xprep[:, :, t0 : t0 + TCH]),
                "x1": np.ascontiguousarray(xprep[:, :, t0 + TCH : t0 + 2 * TCH]),
                "Wp": Wprep,
                "ATp": ATprep,
                "Baug": Baug,
                "ones": np.ones((1, _TC), dtype=_BF16),
            }
        )

    trace = os.environ.get("KERNEL_TRACE", "0") == "1"
    res = run_bass_kernel_spmd(
        nc,
        in_maps,
        core_ids=list(range(_NCORES)),
        trace=trace,
    )
    LAST_RESULT = res

    out = np.empty((_T, _O), dtype=np.float32)
    for c, r in enumerate(res.results):
        out[c * _TC : (c + 1) * _TC, :] = r["out"].T
    return out.reshape(_B, _S, _O)
